# revision 15
# baseline (speedup 1.0000x reference)
"""GCN (2-layer CLIP-GNN) Trainium2 kernel, 8-way SPMD.

Strategy (per spec sharding_hint):
  - Nodes (x / output) sharded across 8 cores; edges partitioned by dst node
    so the segment-sum is core-local; weights replicated.
  - Layer algebra reassociated: A(xW^T)+b == (Ax)W^T+b.  Layer 1 aggregates
    the replicated input x directly (no collective); only the h1 AllGather
    is needed between the layers.
  - Aggregation: edges sorted by dst tile (128 dst nodes per tile).  For each
    tile, dma_gather pulls h[src] rows (edge-major, 128 edges/chunk) into
    SBUF; PE does the segment-sum as matmuls with host-built scatter
    matrices S[e, d] = coef(e) * (dst_local(e)==d), accumulating in PSUM.
  - dma_gather indices are int16 (<32768).  Each core's shard is split into
    two tile-aligned halves A/B; gather sources are host-permuted "all cores'
    A-halves" / "all cores' B-halves" tensors (<=25600 rows each).  The h1
    AllGather splits into AG-A / AG-B: AG-A is issued once the first half of
    layer 1 is done (overlaps the rest of layer 1), and layer 2's A-half
    gathers only wait on AG-A (overlapping AG-B).
"""

import os
import sys
import math
import numpy as np
from contextlib import ExitStack

sys.path.insert(0, "/opt/trn_rl_repo")

import concourse.bass as bass
import concourse.tile as tile
from concourse import mybir, masks, library_config
from concourse.bass_utils import run_bass_kernel_spmd
from concourse.library_overlay import lower_extended_insts

N_CORES = 8
F32 = mybir.dt.float32
BF16 = mybir.dt.bfloat16
F16 = mybir.dt.float16
I16 = mybir.dt.int16

# datapath dtype for gather sources / S / h1 transport / matmul operands
# (PSUM accumulation and the final output stay fp32):
#   "f16"  - half the DMA bytes, full PE rate, ~1e-3 rel err  (default)
#   "bf16" - half the DMA bytes, full PE rate, ~1e-2 rel err
#   "f32"  - exact, but fp32 matmuls run at 1/4 PE rate and 2x DMA bytes
DT = os.environ.get("GNN_DT", "f16")


def dt_pair(name):
    import ml_dtypes
    if name == "f16":
        return F16, np.float16
    if name == "bf16":
        return BF16, ml_dtypes.bfloat16
    return F32, np.float32


# --------------------------------------------------------------------------
# host-side prep: edge partitioning / scatter-matrix construction
# --------------------------------------------------------------------------

def prepare_edges(edge_index, n_nodes, n_cores, gdt_np=np.float32):
    """Partition edges (with self-loops) by dst core / dst 128-tile, split
    by src shard-half (A/B), compute GCN coefs, build per-core S + idx."""
    shard = n_nodes // n_cores
    n_tiles = math.ceil(shard / 128)
    tiles_a = (n_tiles + 1) // 2
    rows_a = min(tiles_a * 128, shard)
    rows_b = shard - rows_a
    assert n_cores * rows_a < 32768 and n_cores * rows_b < 32768

    src = np.concatenate([np.asarray(edge_index[0]), np.arange(n_nodes)]).astype(np.int64)
    dst = np.concatenate([np.asarray(edge_index[1]), np.arange(n_nodes)]).astype(np.int64)
    deg = np.bincount(dst, minlength=n_nodes).astype(np.float64)
    coef = (1.0 / np.sqrt(deg[src] * deg[dst])).astype(np.float32)

    core = dst // shard
    ldst = dst - core * shard
    tile_id = ldst // 128
    d_in_tile = (ldst % 128).astype(np.int64)

    s_core = src // shard
    s_loc = src - s_core * shard
    isb = (s_loc >= rows_a).astype(np.int64)
    # row of src in the A (resp. B) gather-source tensor
    srcrow = np.where(isb == 0, s_core * rows_a + s_loc,
                      s_core * rows_b + (s_loc - rows_a))

    key = (core * n_tiles + tile_id) * 2 + isb
    order = np.argsort(key, kind="stable")
    cnt = np.bincount(key, minlength=n_cores * n_tiles * 2).reshape(n_cores, n_tiles, 2)

    ach = np.ceil(cnt[:, :, 0].max(axis=0) / 128).astype(np.int64)  # per tile
    bch = np.ceil(cnt[:, :, 1].max(axis=0) / 128).astype(np.int64)
    nch = ach + bch
    cs = np.zeros(n_tiles + 1, np.int64)
    cs[1:] = np.cumsum(nch)
    ct = int(cs[-1])

    # rank of each edge within its (core, tile, isb) group
    skey = key[order]
    first = np.zeros(len(skey), np.int64)
    newgrp = np.flatnonzero(np.diff(skey)) + 1
    first[newgrp] = newgrp
    first = np.maximum.accumulate(first)
    rank = np.arange(len(skey)) - first

    o_core = core[order]
    o_tile = tile_id[order]
    o_isb = isb[order]
    o_row = srcrow[order]
    o_d = d_in_tile[order]
    o_coef = coef[order]

    # slot within the tile's chunk run (A slots first, then B)
    slot = rank + o_isb * ach[o_tile] * 128
    chunk = cs[o_tile] + slot // 128
    epart = slot % 128

    S_list, ix_list = [], []
    for p in range(n_cores):
        m = o_core == p
        S = np.zeros((128, ct, 128), gdt_np)
        S[epart[m], chunk[m], o_d[m]] = o_coef[m].astype(gdt_np)
        ix16 = np.zeros((16, ct * 8), np.int16)
        s2 = rank[m]
        callbase = np.where(o_isb[m] == 1, cs[o_tile[m]] + ach[o_tile[m]], cs[o_tile[m]]) * 8
        colidx = callbase + s2 // 16
        rowidx = s2 % 16
        ix16[rowidx, colidx] = o_row[m].astype(np.int16)
        ix_list.append(np.tile(ix16, (8, 1)))  # replicate across the 8 Q7 cores
        S_list.append(S)

    meta = dict(shard=shard, n_tiles=n_tiles, tiles_a=tiles_a, rows_a=rows_a,
                rows_b=rows_b, ach=ach.tolist(), bch=bch.tolist(),
                nch=nch.tolist(), cs=cs.tolist(), ct=ct)
    return S_list, ix_list, meta


def pack_weights(W1, b1, W2, b2, Wc, bc, dt_np=np.float32):
    """Replicated weight tensors in SBUF-ready layouts."""
    D = W1.shape[1]
    H = W1.shape[0]
    C = Wc.shape[0]
    kd, kh = D // 128, H // 128
    w1t = np.ascontiguousarray(W1.T.reshape(kd, 128, H).transpose(1, 0, 2)).astype(dt_np)
    w2t = np.ascontiguousarray(W2.T.reshape(kh, 128, H).transpose(1, 0, 2)).astype(dt_np)
    wct = np.ascontiguousarray(Wc.T.reshape(kh, 128, C).transpose(1, 0, 2)).astype(dt_np)
    return dict(w1t=w1t, w2t=w2t, wct=wct,
                b1=np.asarray(b1).astype(dt_np).reshape(1, H),
                b2=np.asarray(b2).astype(dt_np).reshape(1, H),
                bc=np.asarray(bc).astype(dt_np).reshape(1, C))


# --------------------------------------------------------------------------
# walrus in this container accepts only ONE sync-wait per instruction:
# split multi-wait instructions into single-wait NOPs on the same engine.
# --------------------------------------------------------------------------

def legalize(nc):
    """Post-TileContext fixups needed for this container's walrus."""
    lower_extended_insts(nc)  # populate .instr bytes of extended-inst InstISA
    return split_multi_waits(nc)


def split_multi_waits(nc):
    n_split = 0
    for fn in nc.m.functions:
        for bb in fn.blocks:
            new_insts = []
            for ins in bb.instructions:
                si = ins.sync_info
                if si is not None and si.on_wait and len(si.on_wait) > 1:
                    waits = list(si.on_wait)
                    for w in waits[:-1]:
                        nop = mybir.InstNoOp(name=f"{ins.name}-ws{n_split}", ins=[], outs=[])
                        n_split += 1
                        nop.engine = ins.engine
                        nop.sync_info = mybir.SyncInfo(on_wait=[w], on_update=[])
                        new_insts.append(nop)
                    ins.sync_info = mybir.SyncInfo(on_wait=[waits[-1]],
                                                   on_update=list(si.on_update))
                new_insts.append(ins)
            bb.instructions = new_insts
    return n_split


# --------------------------------------------------------------------------
# device program
# --------------------------------------------------------------------------

def build_program(n_nodes, D, H, C, meta, n_cores=N_CORES, gdt=F32,
                  skip_collectives=False):
    shard = meta["shard"]
    n_tiles, tiles_a = meta["n_tiles"], meta["tiles_a"]
    rows_a, rows_b = meta["rows_a"], meta["rows_b"]
    ach, bch, nch, cs = meta["ach"], meta["bch"], meta["nch"], meta["cs"]
    ct = meta["ct"]
    kd, kh = D // 128, H // 128
    maxch = max(nch)

    nc = bass.Bass("TRN2", target_bir_lowering=False, debug=False, num_devices=n_cores)

    xa_d = nc.dram_tensor("xa", [n_cores * rows_a, D], gdt, kind="ExternalInput")
    xb_d = (nc.dram_tensor("xb", [n_cores * rows_b, D], gdt, kind="ExternalInput")
            if rows_b else None)
    s_d = nc.dram_tensor("S", [128, ct, 128], gdt, kind="ExternalInput")
    ix_d = nc.dram_tensor("ix", [128, ct * 8], I16, kind="ExternalInput")
    w1t_d = nc.dram_tensor("w1t", [128, kd, H], gdt, kind="ExternalInput")
    w2t_d = nc.dram_tensor("w2t", [128, kh, H], gdt, kind="ExternalInput")
    wct_d = nc.dram_tensor("wct", [128, kh, C], gdt, kind="ExternalInput")
    b1_d = nc.dram_tensor("b1", [1, H], gdt, kind="ExternalInput")
    b2_d = nc.dram_tensor("b2", [1, H], gdt, kind="ExternalInput")
    bc_d = nc.dram_tensor("bc", [1, C], gdt, kind="ExternalInput")
    out_d = nc.dram_tensor("out", [shard, C], F32, kind="ExternalOutput")

    h1sa_d = nc.dram_tensor("h1sa", [rows_a, H], gdt)
    h1fa_d = nc.dram_tensor("h1fa", [n_cores * rows_a, H], gdt, addr_space="Shared")
    h1sb_d = nc.dram_tensor("h1sb", [rows_b, H], gdt) if rows_b else None
    h1fb_d = (nc.dram_tensor("h1fb", [n_cores * rows_b, H], gdt, addr_space="Shared")
              if rows_b else None)

    with tile.TileContext(nc) as tc, ExitStack() as ctx:
        const = ctx.enter_context(tc.tile_pool(name="const", bufs=1))
        s_pool = ctx.enter_context(tc.tile_pool(name="spool", bufs=3))
        g_pool = ctx.enter_context(tc.tile_pool(name="gpool", bufs=3))
        zps_pool = ctx.enter_context(tc.tile_pool(name="zps", bufs=2, space="PSUM"))
        tps_pool = ctx.enter_context(tc.tile_pool(name="tps", bufs=2, space="PSUM"))
        hps_pool = ctx.enter_context(tc.tile_pool(name="hps", bufs=2, space="PSUM"))
        zsb_pool = ctx.enter_context(tc.tile_pool(name="zsb", bufs=2))
        zt_pool = ctx.enter_context(tc.tile_pool(name="zt", bufs=2))
        hsb_pool = ctx.enter_context(tc.tile_pool(name="hsb", bufs=3))

        ix_all = const.tile([128, ct * 8], I16)
        nc.sync.dma_start(ix_all[:], ix_d.ap())
        w1t = const.tile([128, kd, H], gdt)
        nc.sync.dma_start(w1t[:], w1t_d.ap())
        w2t = const.tile([128, kh, H], gdt)
        nc.sync.dma_start(w2t[:], w2t_d.ap())
        wct = const.tile([128, kh, C], gdt)
        nc.sync.dma_start(wct[:], wct_d.ap())
        b1 = const.tile([1, H], gdt)
        nc.sync.dma_start(b1[:], b1_d.ap())
        b2 = const.tile([1, H], gdt)
        nc.sync.dma_start(b2[:], b2_d.ap())
        bc = const.tile([1, C], gdt)
        nc.sync.dma_start(bc[:], bc_d.ap())
        ones = const.tile([1, 128], gdt)
        nc.vector.memset(ones[:], 1.0)
        ident = const.tile([128, 128], gdt)
        masks.make_identity(nc, ident[:])
        nc.gpsimd.load_library(library_config.mlp)  # dma_gather lives here

        # one Pool register per distinct gather count (to_reg never caches)
        _regs = {}

        def creg(v):
            if v not in _regs:
                _regs[v] = nc.gpsimd.to_reg(v)
            return _regs[v]

        def transpose_128(dst_sb, src_sb, k_tiles):
            """dst_sb[:, k*128:(k+1)*128] = src_sb[:, k*128:(k+1)*128].T via PE."""
            for k in range(k_tiles):
                tps = tps_pool.tile([128, 128], gdt, tag="tps")
                nc.tensor.transpose(tps[:], src_sb[:, k * 128:(k + 1) * 128], ident[:])
                nc.vector.tensor_copy(dst_sb[:, k * 128:(k + 1) * 128], tps[:])

        def conv_tile(t, src_a_ap, src_b_ap, w_sb, b_sb, k_in, relu, out_dt):
            """One dst tile of aggregate(+transform +bias +opt relu) -> SBUF tile."""
            n_c, c0 = nch[t], cs[t]
            s_tile = s_pool.tile([128, maxch * 128], gdt, tag="s")
            nc.sync.dma_start(s_tile[:, 0:n_c * 128], s_d.ap()[:, c0:c0 + n_c, :])
            g_tile = g_pool.tile([128, maxch, D], gdt, tag="g")
            a_n, b_n = ach[t] * 128, bch[t] * 128
            if a_n:
                nc.gpsimd.dma_gather(
                    g_tile[:, 0:ach[t], :], src_a_ap,
                    ix_all[:, c0 * 8: c0 * 8 + a_n // 16],
                    a_n, creg(a_n), D)
            if b_n:
                nc.gpsimd.dma_gather(
                    g_tile[:, ach[t]:n_c, :], src_b_ap,
                    ix_all[:, c0 * 8 + a_n // 16: c0 * 8 + (a_n + b_n) // 16],
                    b_n, creg(b_n), D)
            zps = zps_pool.tile([128, D], F32, tag="zps")
            for c in range(n_c):
                nc.tensor.matmul(zps[:], s_tile[:, c * 128:(c + 1) * 128],
                                 g_tile[:, c, :], start=(c == 0), stop=(c == n_c - 1))
            zsb = zsb_pool.tile([128, D], gdt, tag="zsb")
            nc.vector.tensor_copy(zsb[:], zps[:])
            zt = zt_pool.tile([128, k_in * 128], gdt, tag="zt")
            transpose_128(zt, zsb, k_in)
            hps = hps_pool.tile([128, w_sb.shape[2]], F32, tag="hps")
            for k in range(k_in):
                nc.tensor.matmul(hps[:], zt[:, k * 128:(k + 1) * 128], w_sb[:, k, :],
                                 start=(k == 0), stop=False)
            nc.tensor.matmul(hps[:], ones[:], b_sb[:], start=False, stop=True)
            hsb = hsb_pool.tile([128, w_sb.shape[2]], out_dt, tag="hsb")
            if relu:
                nc.vector.tensor_scalar_max(hsb[:], hps[:], 0.0)
            else:
                nc.vector.tensor_copy(hsb[:], hps[:])
            return hsb

        xa = xa_d.ap()
        xb = xb_d.ap() if xb_d is not None else None
        ha = h1fa_d.ap()
        hb = h1fb_d.ap() if h1fb_d is not None else None

        # ---- layer 1: h1 = relu((A x) W1^T + b1) -> my h1 shard (A then B half)
        for t in range(n_tiles):
            rows = min(128, shard - t * 128)
            hsb = conv_tile(t, xa, xb, w1t, b1, kd, relu=True, out_dt=gdt)
            if t < tiles_a:
                nc.sync.dma_start(h1sa_d.ap()[t * 128: t * 128 + rows, :], hsb[0:rows, :])
            else:
                r0 = (t - tiles_a) * 128
                nc.sync.dma_start(h1sb_d.ap()[r0: r0 + rows, :], hsb[0:rows, :])
            if t == tiles_a - 1 and not skip_collectives:
                # A half done -> AllGather it while the B half still computes
                nc.gpsimd.collective_compute(
                    "AllGather", mybir.AluOpType.bypass,
                    replica_groups=[list(range(n_cores))],
                    ins=[h1sa_d.ap().opt()], outs=[h1fa_d.ap().opt()])
        if rows_b and not skip_collectives:
            nc.gpsimd.collective_compute(
                "AllGather", mybir.AluOpType.bypass,
                replica_groups=[list(range(n_cores))],
                ins=[h1sb_d.ap().opt()], outs=[h1fb_d.ap().opt()])

        # ---- layer 2 + classifier, fused per tile (h2 never hits DRAM)
        for t in range(n_tiles):
            rows = min(128, shard - t * 128)
            hsb = conv_tile(t, ha, hb, w2t, b2, kh, relu=True, out_dt=gdt)
            h2t = zt_pool.tile([128, kh * 128], gdt, tag="zt")
            transpose_128(h2t, hsb, kh)
            cps = hps_pool.tile([128, C], F32, tag="hps")
            for k in range(kh):
                nc.tensor.matmul(cps[:], h2t[:, k * 128:(k + 1) * 128], wct[:, k, :],
                                 start=(k == 0), stop=False)
            nc.tensor.matmul(cps[:], ones[:], bc[:], start=False, stop=True)
            csb = hsb_pool.tile([128, C], F32, tag="csb")
            nc.vector.tensor_copy(csb[:], cps[:])
            nc.sync.dma_start(out_d.ap()[t * 128: t * 128 + rows, :], csb[0:rows, :])

    return nc


# --------------------------------------------------------------------------
# entry point
# --------------------------------------------------------------------------

def _build_in_maps(x, S_list, ix_list, wpk, meta, n_cores, gdt_np):
    n_nodes, D = x.shape
    shard, rows_a, rows_b = meta["shard"], meta["rows_a"], meta["rows_b"]
    xs = np.asarray(x, np.float32).reshape(n_cores, shard, D)
    xa = np.ascontiguousarray(xs[:, :rows_a].reshape(-1, D)).astype(gdt_np)
    xb = (np.ascontiguousarray(xs[:, rows_a:].reshape(-1, D)).astype(gdt_np)
          if rows_b else None)
    in_maps = []
    for p in range(n_cores):
        m = {"xa": xa, "S": S_list[p], "ix": ix_list[p]}
        if xb is not None:
            m["xb"] = xb
        m.update(wpk)
        in_maps.append(m)
    return in_maps


def kernel(x, edge_index, W1, b1, W2, b2, Wc, bc, _trace=False):
    gdt, gdt_np = dt_pair(DT)

    x = np.asarray(x, np.float32)
    n_nodes, D = x.shape
    H = int(np.asarray(W1).shape[0])
    C = int(np.asarray(Wc).shape[0])

    S_list, ix_list, meta = prepare_edges(edge_index, n_nodes, N_CORES, gdt_np=gdt_np)
    wpk = pack_weights(np.asarray(W1), np.asarray(b1), np.asarray(W2),
                       np.asarray(b2), np.asarray(Wc), np.asarray(bc), dt_np=gdt_np)

    nc = build_program(n_nodes, D, H, C, meta, gdt=gdt)
    legalize(nc)
    in_maps = _build_in_maps(x, S_list, ix_list, wpk, meta, N_CORES, gdt_np)

    res = run_bass_kernel_spmd(nc, in_maps, core_ids=list(range(N_CORES)),
                               trace=_trace)
    out = np.concatenate([res.results[p]["out"] for p in range(N_CORES)], axis=0)
    if _trace:
        return out, res
    return out
